# revision 5
# baseline (speedup 1.0000x reference)
"""sqllm 4-bit LUT-quantized linear: y = x @ dequant(qweight, lut).T
Trainium2 Bass kernel, 8 NeuronCores, column-parallel (shard out_features).

Strategy per core c (N_local = 512 out features):
  - Host: permute x [8192,4096] -> xt [4096,8192] with k'' = j*512 + i ordering
    (k = 8i + j, j = nibble index). This makes on-device nibble-plane
    extraction produce contiguous contraction tiles.
  - Host: qt_c = qweight[:, cN:cN+512].T (n on partitions), and per-(n, hi2)
    cubic coefficients through the 4 LUT points (exact interpolation).
  - Device: dequant W [n, k''] via 3 bit-extracts (GPSIMD) + int->fp convert +
    4 half-cubics (ACT) + 4 fused cubic-tails (custom DVE op) + 3
    copy_predicated merges (DVE). PE-transpose W -> W^T [k'', n].
  - Device: 2048 fp32r matmuls: psum[t128, n512] += xt_tile.T @ W^T_tile,
    accumulated over 32 k-tiles; evacuate to y [8192, 512].
  - Host: concat the 8 cores' y along n.
"""

import numpy as np

import concourse.bass as bass
import concourse.mybir as mybir
import concourse.tile as tile
from concourse import bacc
from concourse.bass_utils import run_bass_kernel_spmd
from concourse.masks import make_identity

# ---------------- problem constants (hardcoded per contract) ---------------- #
B, S, K, N = 4, 2048, 4096, 4096
T = B * S                 # 8192 tokens
NCORES = 8
NL = N // NCORES          # 512 out features per core
KT = K // 128             # 32 contraction tiles
NTT = NL // 128           # 4 n-tiles per core
NPLANES = 8               # nibbles per int32
IW = K // 8               # 512 packed rows
TG = 128                  # tokens per matmul group (one psum tile)
NTG = T // TG             # 64 token groups

F32 = mybir.dt.float32
F32R = mybir.dt.float32r
I32 = mybir.dt.int32

# ---------------- custom DVE op: cubic tail ---------------- #
_CUBIC = None


def _register_cubic_tail():
    """out = s0 + in0*s1 + in0^2 * in1   (s0,s1 per-partition scalars)"""
    global _CUBIC
    if _CUBIC is not None:
        return _CUBIC
    from concourse.dve_ops import DveOp, OPS, CUSTOM_DVE_SPECS, _SUB_OPCODE_FOR_NAME
    from concourse.dve_spec import Spec, Src0, Src1, C0, C1, sq, lower as dve_lower
    from concourse.dve_uop import DveOpSpec

    name = "SQLLM_CUBIC_TAIL"
    if name in _SUB_OPCODE_FOR_NAME:
        _CUBIC = next(op for op in OPS if op.name == name)
        return _CUBIC
    spec = Spec(
        body=C0 + Src0 * C1 + sq(Src0) * Src1,
        reference=lambda in0, in1, s0, s1, imm2: (
            s0 + in0 * s1 + in0 * in0 * in1
        ).astype(np.float32),
    )
    shas = {}
    for ver in ("v3", "v4"):
        tmp = DveOpSpec(name=name, opcode=1, uops=dve_lower(spec, ver=ver), rd1_en=True)
        shas[ver] = tmp.sha(ver)
    op = DveOp(name, spec, subdim=False, uops_sha=shas)
    row = max(_SUB_OPCODE_FOR_NAME.values()) + 1
    assert row < 0x20
    OPS.append(op)
    CUSTOM_DVE_SPECS[name] = spec
    _SUB_OPCODE_FOR_NAME[name] = row
    _CUBIC = op
    return op


# ---------------- device program ---------------- #
def build_nc():
    CUBIC = _register_cubic_tail()
    nc = bacc.Bacc("TRN2", target_bir_lowering=False)
    xt = nc.dram_tensor("xt", [K, T], F32R, kind="ExternalInput")
    qt = nc.dram_tensor("qt", [NL, IW], I32, kind="ExternalInput")
    coef = nc.dram_tensor("coef", [NL, 16], F32, kind="ExternalInput")
    y = nc.dram_tensor("y", [T, NL], F32, kind="ExternalOutput")

    xt_v = xt.rearrange("(s p) t -> p s t", p=128)   # [128, KT, T]
    qt_v = qt.rearrange("(nt p) i -> p nt i", p=128)  # [128, NTT, IW]
    coef_v = coef.rearrange("(nt p) c -> p nt c", p=128)
    y_v = y.rearrange("(tg p) n -> p tg n", p=128)    # [128, NTG, NL]

    with tile.TileContext(nc) as tc:
        with (
            tc.tile_pool(name="persist", bufs=1) as persist,
            tc.tile_pool(name="wt", bufs=1) as wtp,
            tc.tile_pool(name="wn", bufs=6) as wnp,
            tc.tile_pool(name="dq", bufs=2) as dqp,
            tc.tile_pool(name="xb", bufs=2) as xbp,
            tc.tile_pool(name="yb", bufs=3) as ybp,
            tc.tile_pool(name="ps", bufs=4, space="PSUM") as psp,
            tc.tile_pool(name="pst", bufs=2, space="PSUM") as pstp,
        ):
            # persistent: packed weights, coefficients, identity, W^T
            q_sb, c_sb = [], []
            for nt in range(NTT):
                qs = persist.tile([128, IW], I32, tag=f"q{nt}")
                nc.sync.dma_start(qs[:], qt_v[:, nt, :])
                q_sb.append(qs)
                cs = persist.tile([128, 16], F32, tag=f"c{nt}")
                nc.sync.dma_start(cs[:], coef_v[:, nt, :])
                c_sb.append(cs)
            ident = persist.tile([128, 128], F32, tag="ident")
            make_identity(nc, ident[:])
            # W^T stored as 32 k-tiles side by side: [128, KT*512] fp32r
            wt_all = wtp.tile([128, KT * NL], F32R, tag="wt")

            # ---- dequant (j outer so W^T k-tiles become ready in order) ----
            for j in range(NPLANES):
                wn_j = []  # per nt: [128 n, 512 k''-chunk]
                for nt in range(NTT):
                    q = q_sb[nt]
                    lo2i = dqp.tile([128, IW], I32, tag="lo2i")
                    nc.vector.tensor_scalar(
                        out=lo2i[:], in0=q[:], scalar1=4 * j, scalar2=3,
                        op0=mybir.AluOpType.logical_shift_right,
                        op1=mybir.AluOpType.bitwise_and,
                    )
                    bh = dqp.tile([128, IW], I32, tag="bh")
                    nc.vector.tensor_scalar(
                        out=bh[:], in0=q[:], scalar1=4 * j, scalar2=4,
                        op0=mybir.AluOpType.logical_shift_right,
                        op1=mybir.AluOpType.bitwise_and,
                    )
                    BH = dqp.tile([128, IW], I32, tag="BH")
                    nc.vector.tensor_scalar(
                        out=BH[:], in0=q[:], scalar1=4 * j, scalar2=8,
                        op0=mybir.AluOpType.logical_shift_right,
                        op1=mybir.AluOpType.bitwise_and,
                    )
                    lo2f = dqp.tile([128, IW], F32, tag="lo2f")
                    nc.scalar.copy(lo2f[:], lo2i[:])
                    wn = wnp.tile([128, IW], F32, tag=f"wn{nt}")
                    Us = [wn]
                    for g in range(1, 4):
                        Us.append(dqp.tile([128, IW], F32, tag=f"U{g}", name=f"U{g}"))
                    for g in range(4):
                        half = dqp.tile([128, IW], F32, tag="h")
                        nc.scalar.activation(
                            half[:], lo2f[:],
                            mybir.ActivationFunctionType.Identity,
                            bias=c_sb[nt][:, 4 * g + 2: 4 * g + 3],
                            scale=c_sb[nt][:, 4 * g + 3: 4 * g + 4],
                        )
                        nc.vector._custom_dve(
                            CUBIC, out=Us[g][:], in0=lo2f[:], in1=half[:],
                            s0=c_sb[nt][:, 4 * g: 4 * g + 1],
                            s1=c_sb[nt][:, 4 * g + 1: 4 * g + 2],
                        )
                    nc.vector.copy_predicated(Us[0][:], bh[:], Us[1][:])
                    nc.vector.copy_predicated(Us[2][:], bh[:], Us[3][:])
                    nc.vector.copy_predicated(Us[0][:], BH[:], Us[2][:])
                    wn_j.append(wn)
                # ---- transpose this plane's 4 k-tiles: s = 4j + it ----
                for it in range(4):
                    s = 4 * j + it
                    pst = pstp.tile([128, 512], F32)
                    for nt in range(NTT):
                        nc.tensor.transpose(
                            pst[:, nt * 128:(nt + 1) * 128],
                            wn_j[nt][:, it * 128:(it + 1) * 128],
                            ident[:],
                        )
                    nc.vector.tensor_copy(wt_all[:, s * NL:(s + 1) * NL], pst[:])

            # ---- matmul phase ----
            for tg in range(NTG):
                xb = xbp.tile([128, KT * TG], F32R, tag="xb")
                nc.sync.dma_start(
                    xb[:].rearrange("p (s t) -> p s t", s=KT),
                    xt_v[:, :, tg * TG:(tg + 1) * TG],
                )
                ps = psp.tile([128, NL], F32)
                for s in range(KT):
                    nc.tensor.matmul(
                        ps[:],
                        xb[:, s * TG:(s + 1) * TG],
                        wt_all[:, s * NL:(s + 1) * NL],
                        start=(s == 0),
                        stop=(s == KT - 1),
                    )
                yb = ybp.tile([128, NL], F32, tag="yb")
                nc.vector.tensor_copy(yb[:], ps[:])
                nc.sync.dma_start(y_v[:, tg, :], yb[:])
    nc.compile()
    return nc


_NC_CACHE = None


def _get_nc():
    global _NC_CACHE
    if _NC_CACHE is None:
        _NC_CACHE = build_nc()
    return _NC_CACHE


# ---------------- host-side prep ---------------- #
_VINV = np.linalg.inv(np.vander(np.arange(4.0), increasing=True)).astype(np.float64)


def _host_prep(input, qweight, lut):
    x = np.asarray(input, dtype=np.float32).reshape(T, K)
    # k'' = j*512 + i  <->  k = 8i + j
    xt = np.ascontiguousarray(x.reshape(T, IW, 8).transpose(2, 1, 0).reshape(K, T))
    in_maps = []
    for c in range(NCORES):
        nlo = c * NL
        qt_c = np.ascontiguousarray(qweight[:, nlo:nlo + NL].T)  # [NL, IW] int32
        lut_c = np.asarray(lut[nlo:nlo + NL], dtype=np.float64)  # [NL, 16]
        # coef[n, 4g+m]: lut_c[n, 4g+e] = sum_m coef[...m] * e^m
        cf = np.einsum("my,ngy->ngm", _VINV, lut_c.reshape(NL, 4, 4))
        coef_c = np.ascontiguousarray(cf.reshape(NL, 16).astype(np.float32))
        in_maps.append({"xt": xt, "qt": qt_c, "coef": coef_c})
    return in_maps


def kernel(input, qweight, lut):
    nc = _get_nc()
    in_maps = _host_prep(input, qweight, lut)
    res = run_bass_kernel_spmd(nc, in_maps, core_ids=list(range(NCORES)))
    y = np.concatenate([res.results[c]["y"] for c in range(NCORES)], axis=1)
    return np.ascontiguousarray(y.reshape(B, S, N)).astype(np.float32)
